# revision 1
# baseline (speedup 1.0000x reference)
"""Bass/Tile kernel for nn_AttentionBlock (b,t,h,w,c = 1,8,64,64,512) on 8 TRN2 cores.

Sharding: 8 frames (b*t) -> one frame per NeuronCore, weights replicated,
no collectives (attention is independent per frame).

Per-core pipeline:
  Phase 1 (per 512-token block):
    RMSNorm in f32 (per-token factor sqrt(c)/rms; per-channel `scale` folded
    into the QKV weights on host), PE-transpose xn to channel-major f32r,
    QKV matmuls in float32r (full-rate fp32 mode): q,k channel-major
    (1/sqrt(c) folded into Wq/Wk as c^-0.25 each), v token-major bf16.
    q is spilled to DRAM (read once later) to fit SBUF; k,v stay resident.
  Phase 2 (per 128-token q-tile, software-pipelined):
    scores s[q,k] = qt.T @ kT in f32r (PSUM f32), in two 4-bank passes with
    flash-style rescale: rowmax via VectorE, exp via ScalarE with bias=-max
    (accum_out gives the denominator), P in bf16.
    P tiles are PE-transposed to [k,q] for PV (bf16, N=512), the PV output is
    divided by the denominator (per-partition), transposed to channel-major,
    projected, and added to bias + residual.

float32r note: every tensor consumed by an f32r matmul must be *produced*
with dtype float32r (walrus verifier rule), hence the convert-copies.
"""

import numpy as np
import ml_dtypes

B, T, H, W, C = 1, 8, 64, 64, 512
NTOK = H * W          # 4096 tokens per frame
P = 128
TT = NTOK // P        # 32 token tiles
NB = NTOK // 512      # 8 blocks of 512 tokens
CCH = C // P          # 4 channel chunks
EPS = 1e-6
N_CORES = 8

_COMPILED = None
LAST_EXEC_NS = None
TRACE = False


def _build():
    import concourse.bass as bass
    import concourse.tile as tile
    from concourse import mybir, bacc
    from concourse.masks import make_identity

    f32 = mybir.dt.float32
    f32r = mybir.dt.float32r
    bf16 = mybir.dt.bfloat16
    fp16 = mybir.dt.float16
    AF = mybir.ActivationFunctionType
    AX = mybir.AxisListType
    ALU = mybir.AluOpType

    nc = bacc.Bacc()
    x_d = nc.declare_dram_parameter("x", [NTOK, C], f32, isOutput=False)
    wqk_d = nc.declare_dram_parameter("w_qk", [C, 2 * C], f32, isOutput=False)
    wv_d = nc.declare_dram_parameter("w_v", [C, C], f32, isOutput=False)
    wp_d = nc.declare_dram_parameter("w_p", [C, C], bf16, isOutput=False)
    bqk_d = nc.declare_dram_parameter("b_qk", [P, 8], f32, isOutput=False)
    bv_d = nc.declare_dram_parameter("b_v", [P, C], f32, isOutput=False)
    bp_d = nc.declare_dram_parameter("b_p", [P, C], f32, isOutput=False)
    out_d = nc.declare_dram_parameter("out", [NTOK, C], f32, isOutput=True)
    qspill = nc.dram_tensor("qspill", [CCH, P, NTOK], f32)

    from contextlib import ExitStack
    with tile.TileContext(nc) as tc:
        with ExitStack() as ctx:
            consts = ctx.enter_context(tc.tile_pool(name="consts", bufs=1))
            acts = ctx.enter_context(tc.tile_pool(name="acts", bufs=1))
            stagep = ctx.enter_context(tc.tile_pool(name="stage", bufs=1))
            bigp = ctx.enter_context(tc.tile_pool(name="big", bufs=3))
            xin = ctx.enter_context(tc.tile_pool(name="xin", bufs=2))
            sqp = ctx.enter_context(tc.tile_pool(name="sq", bufs=1))
            facp = ctx.enter_context(tc.tile_pool(name="fac", bufs=4))
            xnp = ctx.enter_context(tc.tile_pool(name="xn", bufs=2))
            xnbp = ctx.enter_context(tc.tile_pool(name="xnb", bufs=3))
            qstp = ctx.enter_context(tc.tile_pool(name="qst", bufs=2))
            qtp = ctx.enter_context(tc.tile_pool(name="qt", bufs=2))
            smp = ctx.enter_context(tc.tile_pool(name="sm", bufs=3))
            rdp = ctx.enter_context(tc.tile_pool(name="rd", bufs=3))
            ptp = ctx.enter_context(tc.tile_pool(name="pt", bufs=3))
            atp = ctx.enter_context(tc.tile_pool(name="at", bufs=2))
            xrp = ctx.enter_context(tc.tile_pool(name="xr", bufs=2))
            otp = ctx.enter_context(tc.tile_pool(name="ot", bufs=2))
            pss = ctx.enter_context(tc.tile_pool(name="ps_s", bufs=4, space="PSUM"))
            pst_p = ctx.enter_context(tc.tile_pool(name="ps_t", bufs=2, space="PSUM"))
            pso = ctx.enter_context(tc.tile_pool(name="ps_o", bufs=2, space="PSUM"))
            # ---------- constants / weights ----------
            # wqk in f32r, staged through f32 in halves (f32r matmul inputs
            # must be produced rounded-to-f32r by a compute engine).
            wqk = consts.tile([P, CCH, 2 * C], f32r)
            for h in range(2):
                wst = stagep.tile([P, CCH, C], f32, tag="wst", name=f"wst{h}")
                nc.sync.dma_start(
                    wst, wqk_d[:, h * C:(h + 1) * C].rearrange(
                        "(cc p) d -> p cc d", p=P))
                nc.vector.tensor_copy(wqk[:, :, h * C:(h + 1) * C], wst)
            wv = consts.tile([P, CCH, C], f32r)
            wvst = stagep.tile([P, CCH, C], f32, tag="wst")
            nc.sync.dma_start(wvst, wv_d.rearrange("(cc p) d -> p cc d", p=P))
            nc.vector.tensor_copy(wv, wvst)
            wp = consts.tile([P, CCH, C], bf16)
            nc.sync.dma_start(wp, wp_d.rearrange("(cc p) d -> p cc d", p=P))
            bqk = consts.tile([P, 8], f32)
            nc.sync.dma_start(bqk, bqk_d[:, :])
            bv = consts.tile([P, C], f32)
            nc.sync.dma_start(bv, bv_d[:, :])
            bp = consts.tile([P, C], f32)
            nc.sync.dma_start(bp, bp_d[:, :])
            ident_f = consts.tile([P, P], f32)
            make_identity(nc, ident_f)
            ident_b = consts.tile([P, P], bf16)
            make_identity(nc, ident_b)
            eps_t = consts.tile([P, 1], f32)
            nc.vector.memset(eps_t, EPS / C)

            # ---------- persistent activations ----------
            kT = acts.tile([P, CCH, NTOK], fp16)    # [c_part, c_chunk, tok]
            vv = acts.tile([P, TT, C], bf16)        # [tok_part, tok_tile, c]

            # ---------- phase 1: norm + QKV ----------
            for b in range(NB):
                xnb = xnbp.tile([P, CCH, 512], f32r)   # this block, channel-major
                for t4 in range(4):
                    t = b * 4 + t4
                    xt = xin.tile([P, C], f32)
                    nc.sync.dma_start(xt, x_d[t * P:(t + 1) * P, :])
                    sq = sqp.tile([P, C], f32)
                    ssq = facp.tile([P, 1], f32, tag="ssq")
                    nc.scalar.activation(sq, xt, AF.Square, accum_out=ssq)
                    rmsn = facp.tile([P, 1], f32, tag="rmsn")
                    nc.scalar.activation(rmsn, ssq, AF.Sqrt,
                                         scale=1.0 / (C * C),
                                         bias=eps_t[:, 0:1])
                    fac = facp.tile([P, 1], f32, tag="fac")
                    nc.vector.reciprocal(fac, rmsn)    # sqrt(C)/rms
                    xnt = xnp.tile([P, C], f32)
                    nc.vector.tensor_scalar_mul(xnt, xt, fac)
                    ps = pst_p.tile([P, 512], f32, tag="t")
                    for cc in range(CCH):
                        nc.tensor.transpose(ps[:, cc * P:(cc + 1) * P],
                                            xnt[:, cc * P:(cc + 1) * P],
                                            ident_f)
                    nc.vector.tensor_copy(
                        xnb[:, :, t4 * P:(t4 + 1) * P],
                        ps.rearrange("p (cc j) -> p cc j", cc=CCH))
                # q (spilled to DRAM) and k (resident f32r), channel-major
                for m in range(CCH):
                    pq = pss.tile([P, 512], f32, tag="s", name=f"pq{m}")
                    for cc in range(CCH):
                        nc.tensor.matmul(pq, lhsT=wqk[:, cc, m * P:(m + 1) * P],
                                         rhs=xnb[:, cc, :],
                                         start=(cc == 0), stop=(cc == CCH - 1))
                    qst = qstp.tile([P, 512], f32)
                    nc.scalar.activation(qst, pq, AF.Identity,
                                         bias=bqk[:, m:m + 1])
                    nc.sync.dma_start(
                        qspill[m, :, b * 512:(b + 1) * 512], qst)
                for m in range(CCH):
                    pk = pss.tile([P, 512], f32, tag="s", name=f"pk{m}")
                    for cc in range(CCH):
                        nc.tensor.matmul(
                            pk, lhsT=wqk[:, cc, C + m * P:C + (m + 1) * P],
                            rhs=xnb[:, cc, :],
                            start=(cc == 0), stop=(cc == CCH - 1))
                    nc.scalar.activation(kT[:, m, b * 512:(b + 1) * 512], pk,
                                         AF.Identity, bias=bqk[:, 4 + m:5 + m])
                for t4 in range(4):
                    t = b * 4 + t4
                    pv = pss.tile([P, 512], f32, tag="s", name=f"pv{t4}")
                    for cc in range(CCH):
                        nc.tensor.matmul(pv, lhsT=xnb[:, cc, t4 * P:(t4 + 1) * P],
                                         rhs=wv[:, cc, :],
                                         start=(cc == 0), stop=(cc == CCH - 1))
                    nc.vector.tensor_add(vv[:, t, :], pv, bv)

            # ---------- phase 2: attention + proj, pipelined per q-tile ----
            psb = [None] * TT   # P (softmaxed scores) tiles, bf16 [P, 8, 512]
            rds = [None] * TT   # 1/den per q-tile

            sm_state = {}

            def score_pass(st, kbs):
                qt, mx, dacc, pb, pscore = st
                for kb in kbs:
                    ps = pss.tile([P, 512], f32, tag="s", name=f"ps{kb}")
                    for cc in range(CCH):
                        nc.tensor.matmul(
                            ps, lhsT=qt[:, cc, :],
                            rhs=kT[:, cc, kb * 512:(kb + 1) * 512],
                            start=(cc == 0), stop=(cc == CCH - 1))
                    nc.vector.tensor_reduce(mx[:, kb:kb + 1], ps,
                                            axis=AX.X, op=ALU.max)
                    pscore[kb] = ps

            def exp_pass(st, kbs, negm):
                qt, mx, dacc, pb, pscore = st
                for kb in kbs:
                    nc.scalar.activation(pb[:, kb, :], pscore[kb], AF.Exp,
                                         bias=negm,
                                         accum_out=dacc[:, kb:kb + 1])
                    pscore[kb] = None

            def softmax_A(t):
                qraw = qstp.tile([P, CCH, P], f32, tag="qraw")
                nc.sync.dma_start(
                    qraw,
                    qspill[:, :, t * P:(t + 1) * P].rearrange("m p j -> p m j"))
                qt = qtp.tile([P, CCH, P], fp16)
                nc.vector.tensor_copy(qt, qraw)
                mx = smp.tile([P, 8], f32, tag="mx")
                dacc = smp.tile([P, 8], f32, tag="dacc")
                pb = bigp.tile([P, 8, 512], bf16, tag="big16")
                psb[t] = pb
                st = (qt, mx, dacc, pb, [None] * 8)
                score_pass(st, range(0, 4))
                negA = smp.tile([P, 1], f32, tag="negA")
                nc.vector.tensor_reduce(negA, mx[:, 0:4], axis=AX.X,
                                        op=ALU.max, negate=True)
                exp_pass(st, range(0, 4), negA)
                sm_state[t] = (st, negA)

            def softmax_B(t):
                st, negA = sm_state.pop(t)
                qt, mx, dacc, pb, pscore = st
                score_pass(st, range(4, 8))
                negM = smp.tile([P, 1], f32, tag="negM")
                nc.vector.tensor_reduce(negM, mx[:, 0:8], axis=AX.X,
                                        op=ALU.max, negate=True)
                exp_pass(st, range(4, 8), negM)
                # rescale pass A by exp(mA - m) (=1 when m == mA)
                sdif = smp.tile([P, 1], f32, tag="sdif")
                nc.vector.tensor_tensor(sdif, negM, negA, ALU.subtract)
                scl = smp.tile([P, 1], f32, tag="scl")
                nc.scalar.activation(scl, sdif, AF.Exp)
                nc.vector.tensor_scalar_mul(pb[:, 0:4, :], pb[:, 0:4, :], scl)
                dA = smp.tile([P, 1], f32, tag="dA")
                nc.vector.tensor_reduce(dA, dacc[:, 0:4], axis=AX.X, op=ALU.add)
                dB = smp.tile([P, 1], f32, tag="dB")
                nc.vector.tensor_reduce(dB, dacc[:, 4:8], axis=AX.X, op=ALU.add)
                den = smp.tile([P, 1], f32, tag="den")
                nc.vector.tensor_scalar(den, dA, scalar1=scl, scalar2=dB,
                                        op0=ALU.mult, op1=ALU.add)
                rd = rdp.tile([P, 1], f32)
                nc.vector.reciprocal(rd, den)
                rds[t] = rd

            def tail(t, jrange, po):
                pb = psb[t]
                for j0 in range(jrange.start, jrange.stop, 8):
                    pstt = pst_p.tile([P, 1024], bf16, tag="t", name=f"pt{j0}")
                    for i in range(8):
                        j = j0 + i
                        nc.tensor.transpose(
                            pstt[:, i * P:(i + 1) * P],
                            pb[:, j // 4, (j % 4) * P:(j % 4 + 1) * P],
                            ident_b)
                    ptsb = ptp.tile([P, 1024], bf16)
                    nc.vector.tensor_copy(ptsb, pstt)
                    for i in range(8):
                        j = j0 + i
                        nc.tensor.matmul(po, lhsT=ptsb[:, i * P:(i + 1) * P],
                                         rhs=vv[:, j, :],
                                         start=(j == 0), stop=(j == TT - 1))

            def tail_fin(t, po):
                atok = atp.tile([P, C], bf16, tag="atok")
                nc.vector.tensor_scalar_mul(atok, po, rds[t])
                at = atp.tile([P, CCH, P], bf16, tag="at")
                psat = pst_p.tile([P, 512], bf16, tag="t", name="psat")
                for m in range(CCH):
                    nc.tensor.transpose(psat[:, m * P:(m + 1) * P],
                                        atok[:, m * P:(m + 1) * P],
                                        ident_b)
                nc.vector.tensor_copy(
                    at[:, :, :],
                    psat.rearrange("p (cc j) -> p cc j", cc=CCH))
                pp = pso.tile([P, 512], f32, tag="o", name="pp")
                for m in range(CCH):
                    nc.tensor.matmul(pp, lhsT=at[:, m, :], rhs=wp[:, m, :],
                                     start=(m == 0), stop=(m == CCH - 1))
                xrt = xrp.tile([P, C], f32)
                nc.sync.dma_start(xrt, x_d[t * P:(t + 1) * P, :])
                ott = otp.tile([P, C], f32)
                nc.vector.tensor_add(ott, pp, bp)
                nc.vector.tensor_add(ott, ott, xrt)
                nc.sync.dma_start(out_d[t * P:(t + 1) * P, :], ott)
                psb[t] = None
                rds[t] = None

            for t in range(TT + 1):
                if t < TT:
                    softmax_A(t)
                po = None
                if t >= 1:
                    po = pso.tile([P, 512], f32, tag="o", name=f"po{t}")
                    tail(t - 1, range(0, 16), po)
                if t < TT:
                    softmax_B(t)
                if t >= 1:
                    tail(t - 1, range(16, 32), po)
                    tail_fin(t - 1, po)
    nc.finalize()
    return nc


def _get_nc():
    global _COMPILED
    if _COMPILED is None:
        _COMPILED = _build()
    return _COMPILED


def kernel(x, scale, qkv_w, qkv_b, proj_w, proj_b):
    global LAST_EXEC_NS
    from concourse.bass_utils import run_bass_kernel_spmd

    x = np.asarray(x, dtype=np.float32)
    scale = np.asarray(scale, dtype=np.float32)
    qkv_w = np.asarray(qkv_w, dtype=np.float32)
    qkv_b = np.asarray(qkv_b, dtype=np.float32)
    proj_w = np.asarray(proj_w, dtype=np.float32)
    proj_b = np.asarray(proj_b, dtype=np.float32)

    # host prep: fold `scale` into qkv_w rows; fold attention 1/sqrt(c)
    # (c^-0.25 each) into Wq/Wk and their biases.
    s = C ** -0.25
    w_all = scale[:, None] * qkv_w            # [C, 3C]
    w_q = w_all[:, 0:C] * s
    w_k = w_all[:, C:2 * C] * s
    w_v = np.ascontiguousarray(w_all[:, 2 * C:3 * C], dtype=np.float32)
    b_q = qkv_b[0:C] * s
    b_k = qkv_b[C:2 * C] * s
    b_v = qkv_b[2 * C:3 * C]

    w_qk = np.ascontiguousarray(
        np.concatenate([w_q, w_k], axis=1), dtype=np.float32)
    w_p = proj_w.astype(ml_dtypes.bfloat16)
    b_qk = np.concatenate([b_q.reshape(4, P), b_k.reshape(4, P)], axis=0).T
    b_qk = np.ascontiguousarray(b_qk, dtype=np.float32)
    b_v_b = np.ascontiguousarray(np.broadcast_to(b_v, (P, C)), dtype=np.float32)
    b_p_b = np.ascontiguousarray(np.broadcast_to(proj_b, (P, C)),
                                 dtype=np.float32)

    frames = x.reshape(B * T, NTOK, C)
    in_maps = []
    for i in range(N_CORES):
        in_maps.append({
            "x": np.ascontiguousarray(frames[i]),
            "w_qk": w_qk, "w_v": w_v, "w_p": w_p,
            "b_qk": b_qk, "b_v": b_v_b, "b_p": b_p_b,
        })

    nc = _get_nc()
    res = run_bass_kernel_spmd(nc, in_maps, core_ids=list(range(N_CORES)),
                               trace=TRACE)
    LAST_EXEC_NS = res.exec_time_ns
    out = np.stack([np.asarray(res.results[i]["out"]) for i in range(N_CORES)])
    return out.reshape(B, T, H, W, C).astype(np.float32)

